# revision 13
# baseline (speedup 1.0000x reference)
"""BiGCN (3-layer binarized GCN) on 8 Trainium2 NeuronCores.

Self-contained: hardcodes shapes for
  x[50000,512] f32, edge_index[2,600000] i64, w0[512,128], b0[128],
  w1[128,128], b1[128], w2[128,40], b2[40]  ->  out[50000,40] f32

Node-parallel design, 6250 nodes/core padded to 6272 = 49*128.
Padded global id of node g = (g // 6250) * 6272 + g % 6250.

Per layer:
  h2 = sign(h) @ (sign(w) * mean|w|)      node-major bf16 matmuls
  AllGather h2 -> [50176, 128] bf16 table
  aggregation: self-loop handled by a per-block diagonal matmul against
  the core's own h2 rows; real edges gathered 128-per-tile with
  dma_gather (int16 indices -> table split into lo/hi 25088-row halves;
  tiles are half-pure) and segment-summed by one matmul per tile against
  host-built one-hot segment matrices (gcn norm folded in), accumulated
  per 128-destination block in PSUM.
  Evict: +bias, sign, * dropout mask (host-replicated threefry),
  transpose -> next layer's activations; last layer: log_softmax.

BatchNorm + binactive == sign(x - mean): variance cancels under sign,
so only the column mean is AllReduced.
"""

import numpy as np
import ml_dtypes

from concourse import bass, bacc, mybir
import concourse.tile as tile
from concourse.bass_utils import run_bass_kernel_spmd
from concourse.library_config import mlp as _mlp_lib

N = 50000
E = 600000
IN, HID, OUT = 512, 128, 40
NCORES = 8
SHARD = N // NCORES            # 6250
P = 128
NB = 49                        # dest blocks per core
PADN = NB * P                  # 6272
TBLN = NCORES * PADN           # 50176
HALF = TBLN // 2               # 25088
KIN = IN // P                  # 4
BG = 4                         # dest blocks per gather group
NGRP = (NB + BG - 1) // BG     # 13 groups

F32 = mybir.dt.float32
BF16 = mybir.dt.bfloat16
I16 = mybir.dt.int16
BF16_NP = ml_dtypes.bfloat16


# ---------------------------------------------------------------- host prep

def _dropout_masks():
    """Exact replica of the reference's BernoulliDropout masks ({0,1}).

    Evaluated on the ambient default jax device with the ambient PRNG impl —
    the same way reference() evaluates them when called in this process —
    because the default impl here (rbg) is backend-dependent.
    """
    import jax
    dkey = jax.random.key(42)
    masks = []
    for i in range(2):
        m = jax.random.bernoulli(jax.random.fold_in(dkey, i), 0.5, (N, HID))
        masks.append(np.asarray(m, dtype=np.float32))
    return masks


def _graph_prep(edge_index):
    """Sort real edges by (dest block, table half, source id) per core and
    build, under a schedule shared by all cores (max tiles over cores):
      - the tile stream: groups of BG dest blocks, each group's lo tiles
        then hi tiles, every (block, half) run padded to whole tiles
      - seg[c]  [128, T*128] bf16: per-tile one-hot segment matrices
      - gidx[c] [128, T*8] int16: per-chunk wrapped dma_gather indices
      - segdiag[c] [128, 49*128] bf16: self-loop diagonal blocks
      - chunks: list of (tile_start, n_tiles, half)
      - block_tiles: per block, list of stream tile indices
    """
    ei = np.asarray(edge_index)
    row = ei[0].astype(np.int64)
    col = ei[1].astype(np.int64)
    # degrees including self-loops (reference adds them)
    allr = np.concatenate([row, np.arange(N, dtype=np.int64)])
    deg = np.bincount(allr, minlength=N).astype(np.float64)
    dinv = deg ** -0.5
    norm = (dinv[row] * dinv[col]).astype(np.float32)
    norm_self = (dinv * dinv).astype(np.float32)
    colp = ((col // SHARD) * PADN + col % SHARD).astype(np.int64)

    percore = []
    cnt = np.zeros((NCORES, NB, 2), dtype=np.int64)
    for c in range(NCORES):
        lo, hi = c * SHARD, (c + 1) * SHARD
        sel = (row >= lo) & (row < hi)
        r = (row[sel] - lo).astype(np.int64)
        cp = colp[sel]
        nm = norm[sel]
        blk = r // P
        half = (cp >= HALF).astype(np.int64)
        order = np.lexsort((cp, half, blk))
        r, cp, nm, blk, half = (a[order] for a in (r, cp, nm, blk, half))
        percore.append((r, cp, nm))
        for b in range(NB):
            m = blk == b
            cnt[c, b, 0] = int((half[m] == 0).sum())
            cnt[c, b, 1] = int((half[m] == 1).sum())

    ntile = (cnt.max(axis=0) + P - 1) // P          # [NB, 2] shared schedule
    # stream order: group g -> blocks [g*BG,(g+1)*BG): lo runs then hi runs
    chunks = []                                      # (tile_start, n, half)
    block_tiles = [[] for _ in range(NB)]            # stream tile idx lists
    tile_half = []
    t = 0
    for g in range(NGRP):
        bs = list(range(g * BG, min((g + 1) * BG, NB)))
        for h in (0, 1):
            start = t
            for b in bs:
                for _ in range(int(ntile[b, h])):
                    block_tiles[b].append(t)
                    tile_half.append(h)
                    t += 1
            if t > start:
                chunks.append((start, t - start, h))
    T = t

    seg = np.zeros((NCORES, P, T * P), dtype=BF16_NP)
    gidx = np.zeros((NCORES, P, T * 8), dtype=np.int16)
    segdiag = np.zeros((NCORES, P, NB * P), dtype=BF16_NP)
    for c in range(NCORES):
        r, cp, nm = percore[c]
        blk = r // P
        half = (cp >= HALF).astype(np.int64)
        # per (block, half) contiguous after lexsort
        starts = {}
        keys = blk * 2 + half
        uniq, first = np.unique(keys, return_index=True)
        ends = np.append(first[1:], len(keys))
        for k, f, e in zip(uniq, first, ends):
            starts[int(k)] = (int(f), int(e))
        idx_flat = np.zeros(T * P, dtype=np.int16)
        for b in range(NB):
            tl = block_tiles[b]
            nlo = int(ntile[b, 0])
            for h in (0, 1):
                f, e = starts.get(b * 2 + h, (0, 0))
                cnt_bh = e - f
                tiles_bh = tl[:nlo] if h == 0 else tl[nlo:]
                for ti, st in enumerate(tiles_bh):
                    s0 = f + ti * P
                    s1 = min(f + (ti + 1) * P, e)
                    if s1 <= s0:
                        continue
                    k = s1 - s0
                    sl = np.arange(k)
                    idx_flat[st * P: st * P + k] = (cp[s0:s1] - h * HALF).astype(np.int16)
                    d_loc = (r[s0:s1] - b * P)
                    seg[c, sl, st * P + d_loc] = nm[s0:s1].astype(BF16_NP)
        # wrap indices per chunk: element k -> partition k%16, word k//16,
        # replicated to the 8 groups of 16 partitions
        for (st, n, h) in chunks:
            w = idx_flat[st * P:(st + n) * P].reshape(n * 8, 16).T  # [16, n*8]
            gidx[c, :, st * 8:(st + n) * 8] = np.tile(w, (8, 1))
        # self-loop diagonal
        ar = np.arange(P)
        for b in range(NB):
            rows = b * P + ar
            valid = rows < SHARD
            segdiag[c, ar[valid], b * P + ar[valid]] = (
                norm_self[c * SHARD + rows[valid]].astype(BF16_NP))
    return seg, gidx, segdiag, chunks, block_tiles, ntile, T


# ------------------------------------------------------------- device build

def _build(chunks, block_tiles, T):
    nc = bacc.Bacc(
        "TRN2", target_bir_lowering=False, debug=False,
        enable_asserts=False, num_devices=NCORES,
    )
    DI = [IN, HID, HID]
    DO = [HID, HID, OUT]
    ALPHA_INV = [1.0 / (IN * HID), 1.0 / (HID * HID), 1.0 / (HID * OUT)]
    rg = [list(range(NCORES))]
    MAXT = max(n for _, n, _ in chunks)

    # chunk lookup per stream tile
    tile_chunk = [None] * T
    for ci, (st, n, h) in enumerate(chunks):
        for t in range(st, st + n):
            tile_chunk[t] = (ci, t - st)

    x_in = nc.dram_tensor("x", [SHARD, IN], F32, kind="ExternalInput")
    seg_in = nc.dram_tensor("seg", [P, T * P], BF16, kind="ExternalInput")
    gi_in = nc.dram_tensor("gidx", [P, T * 8], I16, kind="ExternalInput")
    sd_in = nc.dram_tensor("segdiag", [P, NB * P], BF16, kind="ExternalInput")
    m_in = [nc.dram_tensor(f"mask{i}", [P, PADN], BF16, kind="ExternalInput")
            for i in range(2)]
    w_in = [nc.dram_tensor(f"w{i}", [DI[i], DO[i]], F32, kind="ExternalInput")
            for i in range(3)]
    bb_in = [nc.dram_tensor(f"bb{i}", [P, DO[i]], F32, kind="ExternalInput")
             for i in range(3)]
    idf_in = nc.dram_tensor("idf", [P, P], F32, kind="ExternalInput")
    idb_in = nc.dram_tensor("idb", [P, P], BF16, kind="ExternalInput")
    out_t = nc.dram_tensor("out", [SHARD, OUT], F32, kind="ExternalOutput")

    with tile.TileContext(nc) as tc:
        with (
            tc.tile_pool(name="const", bufs=1) as cpool,
            tc.tile_pool(name="wstream", bufs=1) as spool,
            tc.tile_pool(name="evict", bufs=3) as epool,
            tc.tile_pool(name="psum", bufs=4, space="PSUM") as pp,
            tc.tile_pool(name="aggp", bufs=4, space="PSUM") as aggpool,
            tc.tile_pool(name="dram", bufs=1, space="DRAM") as dpool,
        ):
            nc.gpsimd.load_library(_mlp_lib)
            id_f32 = cpool.tile([P, P], F32, tag="idf")
            nc.sync.dma_start(id_f32[:], idf_in.ap())
            id_bf = cpool.tile([P, P], BF16, tag="idb")
            nc.sync.dma_start(id_bf[:], idb_in.ap())
            ones_col = cpool.tile([P, 1], F32, tag="onesc")
            nc.vector.memset(ones_col[:], 1.0)
            ones_row = cpool.tile([1, P], F32, tag="onesr")
            nc.vector.memset(ones_row[:], 1.0)

            # ---- weights prep: sign(w) bf16 + alpha broadcast [P,1]
            wbs, alpha_bc, bias_sb = [], [], []
            for i in range(3):
                kt = DI[i] // P
                wtmp = spool.tile([P, kt * DO[i]], F32, tag="wtmp")
                for k in range(kt):
                    nc.sync.dma_start(
                        wtmp[:, k * DO[i]:(k + 1) * DO[i]],
                        w_in[i][k * P:(k + 1) * P, :])
                ws = cpool.tile([P, kt * DO[i]], BF16, tag=f"wbs{i}", name=f"wbs{i}")
                nc.scalar.activation(ws[:], wtmp[:], mybir.ActivationFunctionType.Sign)
                wabs = spool.tile([P, kt * DO[i]], F32, tag="wabs")
                absacc = spool.tile([P, 1], F32, tag="absacc")
                nc.scalar.activation(
                    wabs[:], wtmp[:], mybir.ActivationFunctionType.Abs,
                    accum_out=absacc[:])
                asum_ps = pp.tile([1, 1], F32, tag="pp")
                nc.tensor.matmul(asum_ps[:], lhsT=ones_col[:], rhs=absacc[:],
                                 start=True, stop=True)
                asum_sb = spool.tile([1, 1], F32, tag="asum")
                nc.vector.tensor_copy(asum_sb[:], asum_ps[:])
                al_ps = pp.tile([P, 1], F32, tag="pp")
                nc.tensor.matmul(al_ps[:], lhsT=ones_row[:], rhs=asum_sb[:],
                                 start=True, stop=True)
                al = cpool.tile([P, 1], F32, tag=f"alpha{i}", name=f"alpha{i}")
                nc.scalar.activation(al[:], al_ps[:],
                                     mybir.ActivationFunctionType.Copy,
                                     scale=ALPHA_INV[i])
                wbs.append(ws)
                alpha_bc.append(al)
                bsb = cpool.tile([P, DO[i]], F32, tag=f"bias{i}", name=f"bias{i}")
                nc.sync.dma_start(bsb[:], bb_in[i].ap())
                bias_sb.append(bsb)

            with tc.tile_pool(name="g0p", bufs=1) as g0pool:
                g0T = g0pool.tile([P, KIN * PADN], BF16, tag="g0T")
                gT = [g0T, None, None]

                # ---- phase A: x load, column sums, mean, sign-transpose
                with tc.tile_pool(name="xres", bufs=1) as xpool:
                    xres = xpool.tile([P, NB * IN], F32, tag="xres")
                    nc.vector.memset(xres[:, (NB - 1) * IN:], 0.0)
                    for t in range(NB - 1):
                        nc.sync.dma_start(
                            xres[:, t * IN:(t + 1) * IN],
                            x_in[t * P:(t + 1) * P, :])
                    nc.sync.dma_start(
                        xres[:SHARD - (NB - 1) * P, (NB - 1) * IN:],
                        x_in[(NB - 1) * P:, :])

                    cs_ps = pp.tile([1, IN], F32, tag="pp")
                    for t in range(NB):
                        nc.tensor.matmul(
                            cs_ps[:], lhsT=ones_col[:],
                            rhs=xres[:, t * IN:(t + 1) * IN],
                            start=(t == 0), stop=(t == NB - 1))
                    cs_sb = spool.tile([1, IN], F32, tag="cs_sb")
                    nc.vector.tensor_copy(cs_sb[:], cs_ps[:])
                    cs_dram = dpool.tile([1, IN], F32, tag="cs_d")
                    nc.sync.dma_start(cs_dram[:], cs_sb[:])
                    cs_all = dpool.tile([1, IN], F32, tag="cs_a", addr_space="Shared")
                    nc.gpsimd.collective_compute(
                        "AllReduce", mybir.AluOpType.add, replica_groups=rg,
                        ins=[cs_dram.opt()], outs=[cs_all.opt()])
                    sums_sb = spool.tile([1, IN], F32, tag="sums")
                    nc.sync.dma_start(sums_sb[:], cs_all[:])
                    negmean = cpool.tile([P, KIN], F32, tag="negmean")
                    for g in range(KIN):
                        nm_ps = pp.tile([P, 1], F32, tag="pp")
                        nc.tensor.transpose(
                            nm_ps[:], in_=sums_sb[:, g * P:(g + 1) * P],
                            identity=id_f32[:1, :1])
                        nc.scalar.activation(
                            negmean[:, g:g + 1], nm_ps[:],
                            mybir.ActivationFunctionType.Copy, scale=-1.0 / N)

                    for t in range(NB):
                        for g in range(KIN):
                            tr_ps = pp.tile([P, P], F32, tag="pp")
                            nc.tensor.transpose(
                                tr_ps[:],
                                in_=xres[:, t * IN + g * P: t * IN + (g + 1) * P],
                                identity=id_f32[:])
                            nc.scalar.activation(
                                g0T[:, g * PADN + t * P: g * PADN + (t + 1) * P],
                                tr_ps[:], mybir.ActivationFunctionType.Sign,
                                bias=negmean[:, g:g + 1])

                # ---- big resident tensors loaded after xres is freed
                with (
                    tc.tile_pool(name="acts", bufs=1) as apool,
                    tc.tile_pool(name="msgp", bufs=2) as mpool,
                ):
                    g1T_t = apool.tile([P, PADN], BF16, tag="g1T")
                    gT[1] = g1T_t
                    g2T_t = apool.tile([P, PADN], BF16, tag="g2T")
                    gT[2] = g2T_t
                    mask_sb = [apool.tile([P, PADN], BF16, tag=f"msk{i}",
                                          name=f"msk{i}") for i in range(2)]
                    for i in range(2):
                        nc.sync.dma_start(mask_sb[i][:], m_in[i].ap())
                    segdiag_sb = apool.tile([P, NB * P], BF16, tag="segdiag")
                    nc.sync.dma_start(segdiag_sb[:], sd_in.ap())
                    gidx_sb = apool.tile([P, T * 8], I16, tag="gidx")
                    nc.sync.dma_start(gidx_sb[:], gi_in.ap())

                    # ---- layers
                    for i in range(3):
                        di, do = DI[i], DO[i]
                        kt = di // P
                        # B1: h2 shard (node-major), resident + sent to DRAM
                        h2own = apool.tile([P, NB * P], BF16, tag="h2own")
                        h2shard = dpool.tile([PADN, P], BF16, tag=f"h2s{i}",
                                             name=f"h2s{i}")
                        for b in range(NB):
                            h2_ps = pp.tile([P, do], F32, tag="pp")
                            for k in range(kt):
                                nc.tensor.matmul(
                                    h2_ps[:],
                                    lhsT=gT[i][:, k * PADN + b * P:
                                               k * PADN + (b + 1) * P],
                                    rhs=wbs[i][:, k * do:(k + 1) * do],
                                    start=(k == 0), stop=(k == kt - 1))
                            if do < P:
                                nc.vector.memset(
                                    h2own[:, b * P + do:(b + 1) * P], 0.0)
                            nc.scalar.activation(
                                h2own[:, b * P: b * P + do], h2_ps[:],
                                mybir.ActivationFunctionType.Copy,
                                scale=alpha_bc[i][:])
                            nc.sync.dma_start(
                                h2shard[b * P:(b + 1) * P, :],
                                h2own[:, b * P:(b + 1) * P])

                        # B2: AllGather the node table
                        h2full = dpool.tile([TBLN, P], BF16, tag=f"h2f{i}",
                                            name=f"h2f{i}", addr_space="Shared")
                        nc.gpsimd.collective_compute(
                            "AllGather", mybir.AluOpType.bypass, replica_groups=rg,
                            ins=[h2shard.opt()], outs=[h2full.opt()])

                        # B3: chunked gathers + segment-sum matmuls + evicts
                        msgs = [None] * len(chunks)
                        segs = [None] * len(chunks)
                        for ci, (st, n, h) in enumerate(chunks):
                            m = mpool.tile([P, MAXT * P], BF16, tag="msg")
                            src = h2full[:HALF, :] if h == 0 else h2full[HALF:, :]
                            nc.gpsimd.dma_gather(
                                m[:, :n * P].rearrange("p (t d) -> p t d", d=P),
                                src, gidx_sb[:, st * 8:(st + n) * 8],
                                n * P, n * P, P, single_packet=False)
                            msgs[ci] = m
                            sgc = mpool.tile([P, MAXT * P], BF16, tag="segc")
                            nc.sync.dma_start(sgc[:, :n * P],
                                              seg_in[:, st * P:(st + n) * P])
                            segs[ci] = sgc

                        for b in range(NB):
                            agg_ps = aggpool.tile([P, do], F32, tag="agg")
                            tl = block_tiles[b]
                            nc.tensor.matmul(
                                agg_ps[:], lhsT=segdiag_sb[:, b * P:(b + 1) * P],
                                rhs=h2own[:, b * P: b * P + do],
                                start=True, stop=(len(tl) == 0))
                            for j, t in enumerate(tl):
                                ci, off = tile_chunk[t]
                                nc.tensor.matmul(
                                    agg_ps[:],
                                    lhsT=segs[ci][:, off * P:(off + 1) * P],
                                    rhs=msgs[ci][:, off * P: off * P + do],
                                    start=False, stop=(j == len(tl) - 1))
                            if i < 2:
                                tmp = epool.tile([P, do], F32, tag="tmp")
                                nc.vector.tensor_add(tmp[:], agg_ps[:],
                                                     bias_sb[i][:])
                                sg = epool.tile([P, do], BF16, tag="sg")
                                nc.scalar.activation(
                                    sg[:], tmp[:],
                                    mybir.ActivationFunctionType.Sign)
                                gm = epool.tile([P, do], BF16, tag="gm")
                                nc.vector.tensor_mul(
                                    gm[:], sg[:],
                                    mask_sb[i][:, b * P:(b + 1) * P])
                                gt_ps = pp.tile([P, P], BF16, tag="pp")
                                nc.tensor.transpose(gt_ps[:], in_=gm[:],
                                                    identity=id_bf[:])
                                nc.vector.tensor_copy(
                                    gT[i + 1][:, b * P:(b + 1) * P], gt_ps[:])
                            else:
                                tmp = epool.tile([P, do], F32, tag="tmp2")
                                nc.vector.tensor_add(tmp[:], agg_ps[:],
                                                     bias_sb[i][:])
                                nmx = epool.tile([P, 1], F32, tag="nmx")
                                nc.vector.tensor_reduce(
                                    nmx[:], tmp[:], axis=mybir.AxisListType.X,
                                    op=mybir.AluOpType.max, negate=True)
                                ex = epool.tile([P, do], F32, tag="ex")
                                sume = epool.tile([P, 1], F32, tag="sume")
                                nc.scalar.activation(
                                    ex[:], tmp[:],
                                    mybir.ActivationFunctionType.Exp,
                                    bias=nmx[:], accum_out=sume[:])
                                lse = epool.tile([P, 1], F32, tag="lse")
                                nc.scalar.activation(
                                    lse[:], sume[:],
                                    mybir.ActivationFunctionType.Ln)
                                o_sb = epool.tile([P, do], F32, tag="osb")
                                nc.vector.tensor_scalar(
                                    o_sb[:], tmp[:], scalar1=nmx[:],
                                    scalar2=lse[:],
                                    op0=mybir.AluOpType.add,
                                    op1=mybir.AluOpType.subtract)
                                rows = P if b < NB - 1 else SHARD - (NB - 1) * P
                                nc.sync.dma_start(
                                    out_t[b * P: b * P + rows, :],
                                    o_sb[:rows, :])

    nc.compile()
    return nc


# ------------------------------------------------------------------ driver

_CACHE = {}


def _get_program(edge_index):
    key = hash(np.asarray(edge_index).tobytes())
    if key not in _CACHE:
        seg, gidx, segdiag, chunks, block_tiles, ntile, T = _graph_prep(edge_index)
        nc = _build(chunks, block_tiles, T)
        _CACHE[key] = (nc, seg, gidx, segdiag)
    return _CACHE[key]


def kernel(x, edge_index, w0, b0, w1, b1, w2, b2, _trace=False):
    nc, seg, gidx, segdiag = _get_program(edge_index)
    masks = _dropout_masks()

    x = np.asarray(x, dtype=np.float32)
    ws = [np.asarray(w, dtype=np.float32) for w in (w0, w1, w2)]
    bs = [np.asarray(b, dtype=np.float32) for b in (b0, b1, b2)]
    ident = np.eye(P, dtype=np.float32)

    in_maps = []
    for c in range(NCORES):
        im = {
            "x": np.ascontiguousarray(x[c * SHARD:(c + 1) * SHARD]),
            "seg": np.ascontiguousarray(seg[c]),
            "gidx": np.ascontiguousarray(gidx[c]),
            "segdiag": np.ascontiguousarray(segdiag[c]),
            "idf": ident,
            "idb": ident.astype(BF16_NP),
        }
        for i in range(2):
            mc = masks[i][c * SHARD:(c + 1) * SHARD]
            mr = np.zeros((PADN, HID), dtype=np.float32)
            mr[:SHARD] = mc
            im[f"mask{i}"] = np.ascontiguousarray(
                mr.reshape(NB, P, HID).transpose(1, 0, 2).reshape(P, NB * HID)
            ).astype(BF16_NP)
        for i in range(3):
            im[f"w{i}"] = ws[i]
            im[f"bb{i}"] = np.ascontiguousarray(
                np.broadcast_to(bs[i], (P, bs[i].shape[0])))
        in_maps.append(im)

    res = run_bass_kernel_spmd(
        nc, in_maps, core_ids=list(range(NCORES)), trace=_trace,
    )
    out = np.concatenate([res.results[c]["out"] for c in range(NCORES)], axis=0)
    if _trace:
        kernel.last_exec_time_ns = res.exec_time_ns
        kernel.last_results = res
    return out
